# revision 18
# baseline (speedup 1.0000x reference)
"""Trainium2 8-core GCN kernel (nn_Net_171798692309).

3-layer GCNConv (1700->256->256->256) + global mean pool + linear head.

Strategy:
  - Nodes sharded contiguously across 8 cores (2500/core, padded to 2560).
  - Edges (incl. self-loops) partitioned by TARGET node; per 128-target-node
    tile, edge lists padded to CMAX chunks of 128 edges.
  - x@W1 computed per-core from a host-pretransposed fp16 x^T slice.
  - Feature rows exchanged via AllGather (the "halo exchange" - random graph
    => halo == everything).
  - Per conv: dma_gather pulls 128-row message blocks [128e, 256f] from the
    all-gathered features in HBM; scatter-add is a one-hot matmul
    (P[e, n] = norm_e at column (target-base)) accumulating in PSUM.
  - Conv3 collapses algebraically: its output only feeds a global mean, so
    mean(conv3(h2)) == (s @ h2) @ W3 / N + b3 with s[n] = sum of norm over
    edges with source n. No third gather/scatter.
  - fp16 storage / fp32 accumulation (host-validated rel err ~5e-4).
"""

import sys

import numpy as np

sys.path.insert(0, "/opt/trn_rl_repo")

from concourse import bacc, bass, bass_utils, mybir, tile  # noqa: E402

N = 20000
F = 1700
FPAD = 1792  # 14*128
H = 256
NCORES = 8
NPC = N // NCORES  # 2500 nodes per core
NT = 20  # 128-node tiles per core (last tile: 68 real nodes)
NTP = NT * 128  # 2560 padded nodes per core
NFC = FPAD // 128  # 14 f-chunks for layer 1

f16 = mybir.dt.float16
f32 = mybir.dt.float32
i16 = mybir.dt.int16


def _prep(x, edge_index, edge_attr, W1, b1, W2, b2, W3, b3, Wl, bl):
    """Host-side graph preprocessing + input packing. Returns (in_maps, CMAX)."""
    x = np.asarray(x, np.float32)
    ei = np.asarray(edge_index).astype(np.int64)
    ew = np.asarray(edge_attr, np.float32)
    row, col = ei[0], ei[1]
    E = row.shape[0]

    # symmetric-normalized adjacency weights (same math as reference gcn_conv)
    deg = np.zeros(N, np.float32)
    np.add.at(deg, col, ew)
    deg += 1.0  # self loop weight
    dis = (1.0 / np.sqrt(deg)).astype(np.float32)
    loops = np.arange(N, dtype=np.int64)
    r_all = np.concatenate([row, loops])
    c_all = np.concatenate([col, loops])
    n_all = np.concatenate(
        [(dis[row] * ew * dis[col]).astype(np.float32), (1.0 / deg).astype(np.float32)]
    )
    # s[n] = sum of norm over edges with source n (for collapsed conv3)
    s = np.zeros(N, np.float32)
    np.add.at(s, r_all, n_all)

    # partition edges by target core / target tile
    core = c_all // NPC
    tl = (c_all % NPC) // 128
    coff = (c_all % NPC) % 128
    gid = (core * NT + tl).astype(np.int64)
    order = np.lexsort((r_all, gid))  # by tile, then by source (HBM locality)
    gid_s = gid[order]
    r_s = r_all[order]
    n_s = n_all[order]
    coff_s = coff[order]
    counts = np.bincount(gid_s, minlength=NCORES * NT)
    CMAX = max(1, int(np.ceil(counts.max() / 128.0)))
    starts = np.concatenate([[0], np.cumsum(counts)[:-1]])
    j = np.arange(E + N) - starts[gid_s]  # rank within (core,tile)
    core_s = gid_s // NT
    t_s = gid_s % NT

    # gather index arrays: item j of tile t lives at [j%16, t*CMAX*8 + j//16],
    # replicated across the 8 Q7 cores (partition stripes 16k..16k+15)
    idx16 = np.zeros((NCORES, 16, NT * CMAX * 8), np.int16)
    idx16[core_s, j % 16, t_s * (CMAX * 8) + j // 16] = r_s.astype(np.int16)
    idx_arr = np.tile(idx16, (1, 8, 1))
    # one-hot norm matrices: P[p=j%128, (t*CMAX + j//128)*128 + coff] = norm
    P_arr = np.zeros((NCORES, 128, NT * CMAX * 128), np.float16)
    P_arr[core_s, j % 128, (t_s * CMAX + j // 128) * 128 + coff_s] = n_s.astype(
        np.float16
    )

    # s packed [core, p, t]
    s_pad = np.zeros(NCORES * NTP, np.float32)
    s_pad.reshape(NCORES, NTP)[:, :NPC] = s.reshape(NCORES, NPC)
    s_arr = np.ascontiguousarray(
        s_pad.reshape(NCORES, NT, 128).transpose(0, 2, 1).astype(np.float16)
    )

    # per-core x^T slices, padded [FPAD, NTP] fp16
    xT_arr = np.zeros((NCORES, FPAD, NTP), np.float16)
    for c in range(NCORES):
        xc = x[c * NPC : (c + 1) * NPC]  # [2500, 1700]
        xT_arr[c, :F, :NPC] = xc.T.astype(np.float16)

    W1p = np.zeros((FPAD, H), np.float32)
    W1p[:F] = np.asarray(W1, np.float32)
    W1pk = np.ascontiguousarray(
        W1p.reshape(NFC, 128, H).transpose(1, 0, 2).astype(np.float16)
    )
    W2pk = np.ascontiguousarray(
        np.asarray(W2, np.float32).reshape(2, 128, H).transpose(1, 0, 2).astype(np.float16)
    )
    W3pk = np.ascontiguousarray(
        np.asarray(W3, np.float32)
        .reshape(2, 128, 2, 128)
        .transpose(1, 0, 2, 3)
        .astype(np.float16)
    )
    Wlpk = np.ascontiguousarray(
        np.asarray(Wl, np.float32).reshape(2, 128, F).transpose(1, 0, 2).astype(np.float16)
    )
    b1T = np.ascontiguousarray(np.asarray(b1, np.float32).reshape(2, 128).T)
    b2rep = np.ascontiguousarray(
        np.tile(np.asarray(b2, np.float32)[None, :], (128, 1))
    )
    b3T = np.ascontiguousarray(np.asarray(b3, np.float32).reshape(2, 128).T)
    blr = np.asarray(bl, np.float32).reshape(1, F)

    shared = {
        "W1": W1pk,
        "W2": W2pk,
        "W3": W3pk,
        "Wl": Wlpk,
        "b1": b1T,
        "b2": b2rep,
        "b3": b3T,
        "bl": blr,
    }
    in_maps = []
    for c in range(NCORES):
        m = dict(shared)
        m["xT"] = np.ascontiguousarray(xT_arr[c])
        m["Pm"] = np.ascontiguousarray(P_arr[c])
        m["idx"] = np.ascontiguousarray(idx_arr[c])
        m["sw"] = s_arr[c]
        in_maps.append(m)
    return in_maps, CMAX


def _build(CMAX, phases="ABCDE"):
    """Build the SPMD bass program (identical on all 8 cores).

    phases may carry a "@R" suffix: phases B and D are then emitted R times
    (timing amplification for wall-clock differencing; results unchanged).
    """
    REP = 1
    if "@" in phases:
        phases, r = phases.split("@")
        REP = int(r)
    nc = bacc.Bacc(
        "TRN2",
        target_bir_lowering=False,
        debug=False,
        enable_asserts=False,
        num_devices=NCORES,
    )
    xT_d = nc.dram_tensor("xT", [FPAD, NTP], f16, kind="ExternalInput")
    Pm_d = nc.dram_tensor("Pm", [128, NT * CMAX * 128], f16, kind="ExternalInput")
    idx_d = nc.dram_tensor("idx", [128, NT * CMAX * 8], i16, kind="ExternalInput")
    sw_d = nc.dram_tensor("sw", [128, NT], f16, kind="ExternalInput")
    W1_d = nc.dram_tensor("W1", [128, NFC, H], f16, kind="ExternalInput")
    W2_d = nc.dram_tensor("W2", [128, 2, H], f16, kind="ExternalInput")
    W3_d = nc.dram_tensor("W3", [128, 2, 2, 128], f16, kind="ExternalInput")
    Wl_d = nc.dram_tensor("Wl", [128, 2, F], f16, kind="ExternalInput")
    b1_d = nc.dram_tensor("b1", [128, 2], f32, kind="ExternalInput")
    b2_d = nc.dram_tensor("b2", [128, H], f32, kind="ExternalInput")
    b3_d = nc.dram_tensor("b3", [128, 2], f32, kind="ExternalInput")
    bl_d = nc.dram_tensor("bl", [1, F], f32, kind="ExternalInput")
    out_d = nc.dram_tensor("out", [1, F], f32, kind="ExternalOutput")

    rg = [list(range(NCORES))]
    NSPAN = 4
    SPAN = NTP // NSPAN  # 640 nodes per x^T staging span

    with tile.TileContext(nc) as tc:
        with (
            tc.tile_pool(name="const", bufs=1) as cp,
            tc.tile_pool(name="xts", bufs=2) as xtp,
            tc.tile_pool(name="gat", bufs=3) as gp,
            tc.tile_pool(name="ff", bufs=3) as fp,
            tc.tile_pool(name="dram", bufs=1, space=bass.MemorySpace.DRAM) as dp,
            tc.tile_pool(name="ps_main", bufs=2, space=bass.MemorySpace.PSUM) as ps_main,
            tc.tile_pool(name="ps_ab", bufs=4, space=bass.MemorySpace.PSUM) as ps_ab,
            tc.tile_pool(name="ps_t", bufs=2, space=bass.MemorySpace.PSUM) as ps_t,
        ):
            # ---- resident constants -> SBUF
            P_sb = cp.tile([128, NT * CMAX * 128], f16)
            nc.sync.dma_start(P_sb[:], Pm_d[:, :])
            idx_sb = cp.tile([128, NT * CMAX * 8], i16)
            nc.sync.dma_start(idx_sb[:], idx_d[:, :])
            s_sb = cp.tile([128, NT], f16)
            nc.sync.dma_start(s_sb[:], sw_d[:, :])
            W1_sb = cp.tile([128, NFC, H], f16)
            nc.sync.dma_start(W1_sb[:], W1_d[:, :, :])
            W2_sb = cp.tile([128, 2, H], f16)
            nc.sync.dma_start(W2_sb[:], W2_d[:, :, :])
            W3_sb = cp.tile([128, 2, 2, 128], f16)
            nc.sync.dma_start(W3_sb[:], W3_d[:, :, :, :])
            Wl_sb = cp.tile([128, 2, F], f16)
            nc.sync.dma_start(Wl_sb[:], Wl_d[:, :, :])
            b1_sb = cp.tile([128, 2], f32)
            nc.sync.dma_start(b1_sb[:], b1_d[:, :])
            b2_sb = cp.tile([128, H], f32)
            nc.sync.dma_start(b2_sb[:], b2_d[:, :])
            b3_sb = cp.tile([128, 2], f32)
            nc.sync.dma_start(b3_sb[:], b3_d[:, :])
            bl_sb = cp.tile([1, F], f32)
            nc.sync.dma_start(bl_sb[:], bl_d[:, :])

            h1T_sb = cp.tile([128, 2, NTP], f16)  # h1^T (features on partitions)

            # DRAM bounce/result buffers
            x1_local = dp.tile([NTP, H], f16)
            x1_full = dp.tile([N, H], f16)
            x2_local = dp.tile([NTP, H], f16)
            x2_full = dp.tile([N, H], f16)
            t_local = dp.tile([128, 2], f32)
            t_red = dp.tile([128, 2], f32)

            # ================= Phase A: x1 = x @ W1 (node-major out) ======
            x1l_ap = x1_local[:, :].rearrange("(t p) f -> p t f", p=128)
            for sp in range(NSPAN):
                xts = xtp.tile([128, NFC, SPAN], f16)
                nc.sync.dma_start(
                    xts[:], xT_d[:, :].rearrange("(c p) n -> p c n", p=128)[
                        :, :, sp * SPAN : (sp + 1) * SPAN
                    ]
                )
                for tt in range(SPAN // 128):
                    t = sp * (SPAN // 128) + tt
                    ps = ps_main.tile([128, H], f32, tag="ps")
                    for fc in range(NFC):
                        nc.tensor.matmul(
                            ps[:],
                            xts[:, fc, tt * 128 : (tt + 1) * 128],
                            W1_sb[:, fc, :],
                            start=(fc == 0),
                            stop=(fc == NFC - 1),
                        )
                    x1t = fp.tile([128, H], f16, tag="xf")
                    nc.vector.tensor_copy(x1t[:], ps[:])
                    nc.sync.dma_start(x1l_ap[:, t, :], x1t[:])

            # ================= AllGather x1 ===============================
            if "G" not in phases:  # G = skip collectives (debug)
                nc.gpsimd.collective_compute(
                    "AllGather",
                    mybir.AluOpType.bypass,
                    replica_groups=rg,
                    ins=[x1_local[0:NPC, :].opt()],
                    outs=[x1_full[:, :].opt()],
                )
            else:
                nc.sync.dma_start(x1_full[0:NTP, :], x1_local[:, :])

            # ================= Phase B: conv1 aggregation (h1^T out) ======
            for t in range((NT * REP) if "B" in phases else 0):
                t = t % NT
                gat = gp.tile([128, CMAX, H], f16, tag="gat")
                nc.gpsimd.dma_gather(
                    gat[:],
                    x1_full[:, :],
                    idx_sb[:, t * CMAX * 8 : (t + 1) * CMAX * 8],
                    num_idxs=CMAX * 128,
                    num_idxs_reg=CMAX * 128,
                    elem_size=H,
                    single_packet=False,
                )
                psA = ps_ab.tile([128, 128], f32, tag="psab")
                psB = ps_ab.tile([128, 128], f32, tag="psab")
                for c in range(CMAX):
                    pc = P_sb[:, (t * CMAX + c) * 128 : (t * CMAX + c + 1) * 128]
                    nc.tensor.matmul(
                        psA[:], gat[:, c, 0:128], pc,
                        start=(c == 0), stop=(c == CMAX - 1),
                    )
                    nc.tensor.matmul(
                        psB[:], gat[:, c, 128:256], pc,
                        start=(c == 0), stop=(c == CMAX - 1),
                    )
                nc.scalar.activation(
                    h1T_sb[:, 0, t * 128 : (t + 1) * 128], psA[:],
                    mybir.ActivationFunctionType.Relu, bias=b1_sb[:, 0:1],
                )
                nc.scalar.activation(
                    h1T_sb[:, 1, t * 128 : (t + 1) * 128], psB[:],
                    mybir.ActivationFunctionType.Relu, bias=b1_sb[:, 1:2],
                )

            # ================= Phase C: x2 = h1 @ W2 ======================
            x2l_ap = x2_local[:, :].rearrange("(t p) f -> p t f", p=128)
            for t in range(NT if "C" in phases else 0):
                ps = ps_main.tile([128, H], f32, tag="ps")
                for hh in range(2):
                    nc.tensor.matmul(
                        ps[:],
                        h1T_sb[:, hh, t * 128 : (t + 1) * 128],
                        W2_sb[:, hh, :],
                        start=(hh == 0),
                        stop=(hh == 1),
                    )
                x2t = fp.tile([128, H], f16, tag="xf")
                nc.vector.tensor_copy(x2t[:], ps[:])
                nc.sync.dma_start(x2l_ap[:, t, :], x2t[:])

            # ================= AllGather x2 ===============================
            if "D" in phases and "G" not in phases:
                nc.gpsimd.collective_compute(
                    "AllGather",
                    mybir.AluOpType.bypass,
                    replica_groups=rg,
                    ins=[x2_local[0:NPC, :].opt()],
                    outs=[x2_full[:, :].opt()],
                )
            elif "D" in phases:
                nc.sync.dma_start(x2_full[0:NTP, :], x2_local[:, :])

            # ====== Phase D: conv2 aggregation (node-major) + s-fold ======
            tpsA = ps_t.tile([128, 1], f32, tag="tps")
            tpsB = ps_t.tile([128, 1], f32, tag="tps")
            for tr in range((NT * REP) if "D" in phases else 0):
                rep, t = divmod(tr, NT)
                gat = gp.tile([128, CMAX, H], f16, tag="gat")
                nc.gpsimd.dma_gather(
                    gat[:],
                    x2_full[:, :],
                    idx_sb[:, t * CMAX * 8 : (t + 1) * CMAX * 8],
                    num_idxs=CMAX * 128,
                    num_idxs_reg=CMAX * 128,
                    elem_size=H,
                    single_packet=False,
                )
                ps2 = ps_main.tile([128, H], f32, tag="ps")
                for c in range(CMAX):
                    nc.tensor.matmul(
                        ps2[:],
                        P_sb[:, (t * CMAX + c) * 128 : (t * CMAX + c + 1) * 128],
                        gat[:, c, :],
                        start=(c == 0),
                        stop=(c == CMAX - 1),
                    )
                h2t = fp.tile([128, H], f16, tag="h2")
                nc.vector.tensor_tensor(h2t[:], ps2[:], b2_sb[:], mybir.AluOpType.add)
                nc.vector.tensor_relu(h2t[:], h2t[:])
                if rep != REP - 1:
                    continue
                # t += h2_tile^T @ s_tile  (accumulated across all 20 tiles)
                nc.tensor.matmul(
                    tpsA[:], h2t[:, 0:128], s_sb[:, t : t + 1],
                    start=(t == 0), stop=(t == NT - 1), skip_group_check=True,
                )
                nc.tensor.matmul(
                    tpsB[:], h2t[:, 128:256], s_sb[:, t : t + 1],
                    start=(t == 0), stop=(t == NT - 1), skip_group_check=True,
                )

            # ================= Phase E: head ==============================
            if "E" not in phases:
                outsb0 = cp.tile([1, F], f32)
                nc.gpsimd.memset(outsb0[:], 0.0)
                nc.sync.dma_start(out_d[:, :], outsb0[:])
            else:
                tsb = cp.tile([128, 2], f32)
                nc.vector.tensor_copy(tsb[:, 0:1], tpsA[:])
                nc.vector.tensor_copy(tsb[:, 1:2], tpsB[:])
                nc.sync.dma_start(t_local[:, :], tsb[:])
                if "G" not in phases:
                    nc.gpsimd.collective_compute(
                        "AllReduce",
                        mybir.AluOpType.add,
                        replica_groups=rg,
                        ins=[t_local[:, :].opt()],
                        outs=[t_red[:, :].opt()],
                    )
                else:
                    nc.sync.dma_start(t_red[:, :], t_local[:, :])
                t16 = cp.tile([128, 2], f16)
                nc.gpsimd.dma_start(t16[:], t_red[:, :])  # SWDGE cast f32->f16

                g16 = cp.tile([128, 2], f16)
                for o in range(2):
                    psg = ps_t.tile([128, 1], f32, tag="tps")
                    for i in range(2):
                        nc.tensor.matmul(
                            psg[:], W3_sb[:, i, o, :], t16[:, i : i + 1],
                            start=(i == 0), stop=(i == 1),
                        )
                    # g = psg / N + b3  (cast to fp16)
                    nc.vector.tensor_scalar(
                        g16[:, o : o + 1], psg[:], 1.0 / N, b3_sb[:, o : o + 1],
                        mybir.AluOpType.mult, mybir.AluOpType.add,
                    )

                outsb = cp.tile([1, F], f32)
                off = 0
                while off < F:
                    w = min(512, F - off)
                    pso = ps_main.tile([1, 512], f32, tag="ps")
                    for hh in range(2):
                        nc.tensor.matmul(
                            pso[0:1, 0:w], g16[:, hh : hh + 1],
                            Wl_sb[:, hh, off : off + w],
                            start=(hh == 0), stop=(hh == 1),
                        )
                    nc.vector.tensor_tensor(
                        outsb[0:1, off : off + w], pso[0:1, 0:w],
                        bl_sb[0:1, off : off + w], mybir.AluOpType.add,
                    )
                    off += w
                nc.sync.dma_start(out_d[:, :], outsb[:])

    nc.compile()
    return nc


_CACHE = {}


def _get_nc(CMAX, phases="ABCDE"):
    key = (CMAX, phases)
    if key not in _CACHE:
        _CACHE[key] = _build(CMAX, phases)
    return _CACHE[key]


def kernel(x, edge_index, edge_attr, W1, b1, W2, b2, W3, b3, Wl, bl, **kw):
    in_maps, CMAX = _prep(x, edge_index, edge_attr, W1, b1, W2, b2, W3, b3, Wl, bl)
    nc = _get_nc(CMAX)
    res = bass_utils.run_bass_kernel_spmd(nc, in_maps, core_ids=list(range(NCORES)))
    return np.asarray(res.results[0]["out"], np.float32).reshape(1, F)


if __name__ == "__main__":
    import reference

    inputs = {k: np.asarray(v) for k, v in reference.setup_inputs().items()}
    out = kernel(**inputs)
    print("out", out.shape, out.dtype, out[0, :5])


# revision 23
# speedup vs baseline: 2.0232x; 2.0232x over previous
"""Trainium2 8-core GCN kernel (nn_Net_171798692309).

3-layer GCNConv (1700->256->256->256) + global mean pool + linear head.

Strategy:
  - Nodes sharded contiguously across 8 cores (2500/core, padded to 2560).
  - Edges (incl. self-loops) partitioned by TARGET node; per 128-target-node
    tile, edge lists padded to CMAX chunks of 128 edges.
  - x@W1 computed per-core from a host-pretransposed fp16 x^T slice.
  - Feature rows exchanged via AllGather (the "halo exchange" - random graph
    => halo == everything).
  - Per conv: dma_gather pulls 128-row message blocks [128e, 256f] from the
    all-gathered features in HBM; scatter-add is a one-hot matmul
    (P[e, n] = norm_e at column (target-base)) accumulating in PSUM.
  - Conv3 collapses algebraically: its output only feeds a global mean, so
    mean(conv3(h2)) == (s @ h2) @ W3 / N + b3 with s[n] = sum of norm over
    edges with source n. No third gather/scatter.
  - fp16 storage / fp32 accumulation (host-validated rel err ~5e-4).
"""

import sys

import numpy as np

sys.path.insert(0, "/opt/trn_rl_repo")

from concourse import bacc, bass, bass_utils, mybir, tile  # noqa: E402

N = 20000
F = 1700
FPAD = 1792  # 14*128
H = 256
NCORES = 8
NPC = N // NCORES  # 2500 nodes per core
NT = 20  # 128-node tiles per core (last tile: 68 real nodes)
NTP = NT * 128  # 2560 padded nodes per core
NFC = FPAD // 128  # 14 f-chunks for layer 1

f16 = mybir.dt.float16
f32 = mybir.dt.float32
i16 = mybir.dt.int16


def _prep(x, edge_index, edge_attr, W1, b1, W2, b2, W3, b3, Wl, bl):
    """Host-side graph preprocessing + input packing. Returns (in_maps, CMAX)."""
    x = np.asarray(x, np.float32)
    ei = np.asarray(edge_index).astype(np.int64)
    ew = np.asarray(edge_attr, np.float32)
    row, col = ei[0], ei[1]
    E = row.shape[0]

    # symmetric-normalized adjacency weights (same math as reference gcn_conv)
    deg = np.zeros(N, np.float32)
    np.add.at(deg, col, ew)
    deg += 1.0  # self loop weight
    dis = (1.0 / np.sqrt(deg)).astype(np.float32)
    loops = np.arange(N, dtype=np.int64)
    r_all = np.concatenate([row, loops])
    c_all = np.concatenate([col, loops])
    n_all = np.concatenate(
        [(dis[row] * ew * dis[col]).astype(np.float32), (1.0 / deg).astype(np.float32)]
    )
    # s[n] = sum of norm over edges with source n (for collapsed conv3)
    s = np.zeros(N, np.float32)
    np.add.at(s, r_all, n_all)

    # partition edges by target core / target tile
    core = c_all // NPC
    tl = (c_all % NPC) // 128
    coff = (c_all % NPC) % 128
    gid = (core * NT + tl).astype(np.int64)
    order = np.lexsort((r_all, gid))  # by tile, then by source (HBM locality)
    gid_s = gid[order]
    r_s = r_all[order]
    n_s = n_all[order]
    coff_s = coff[order]
    counts = np.bincount(gid_s, minlength=NCORES * NT)
    CMAX = max(1, int(np.ceil(counts.max() / 128.0)))
    starts = np.concatenate([[0], np.cumsum(counts)[:-1]])
    j = np.arange(E + N) - starts[gid_s]  # rank within (core,tile)
    core_s = gid_s // NT
    t_s = gid_s % NT

    # gather index arrays: item j of tile t lives at [j%16, t*CMAX*8 + j//16],
    # replicated across the 8 Q7 cores (partition stripes 16k..16k+15)
    idx16 = np.zeros((NCORES, 16, NT * CMAX * 8), np.int16)
    idx16[core_s, j % 16, t_s * (CMAX * 8) + j // 16] = r_s.astype(np.int16)
    idx_arr = np.tile(idx16, (1, 8, 1))
    # one-hot norm matrices: P[p=j%128, (t*CMAX + j//128)*128 + coff] = norm
    P_arr = np.zeros((NCORES, 128, NT * CMAX * 128), np.float16)
    P_arr[core_s, j % 128, (t_s * CMAX + j // 128) * 128 + coff_s] = n_s.astype(
        np.float16
    )

    # s packed [core, p, t]
    s_pad = np.zeros(NCORES * NTP, np.float32)
    s_pad.reshape(NCORES, NTP)[:, :NPC] = s.reshape(NCORES, NPC)
    s_arr = np.ascontiguousarray(
        s_pad.reshape(NCORES, NT, 128).transpose(0, 2, 1).astype(np.float16)
    )

    # per-core x^T slices, padded [FPAD, NTP] fp16
    xT_arr = np.zeros((NCORES, FPAD, NTP), np.float16)
    for c in range(NCORES):
        xc = x[c * NPC : (c + 1) * NPC]  # [2500, 1700]
        xT_arr[c, :F, :NPC] = xc.T.astype(np.float16)

    W1p = np.zeros((FPAD, H), np.float32)
    W1p[:F] = np.asarray(W1, np.float32)
    W1pk = np.ascontiguousarray(
        W1p.reshape(NFC, 128, H).transpose(1, 0, 2).astype(np.float16)
    )
    W2pk = np.ascontiguousarray(
        np.asarray(W2, np.float32).reshape(2, 128, H).transpose(1, 0, 2).astype(np.float16)
    )
    W3pk = np.ascontiguousarray(
        np.asarray(W3, np.float32)
        .reshape(2, 128, 2, 128)
        .transpose(1, 0, 2, 3)
        .astype(np.float16)
    )
    Wlpk = np.ascontiguousarray(
        np.asarray(Wl, np.float32).reshape(2, 128, F).transpose(1, 0, 2).astype(np.float16)
    )
    b1T = np.ascontiguousarray(np.asarray(b1, np.float32).reshape(2, 128).T)
    b2rep = np.ascontiguousarray(
        np.tile(np.asarray(b2, np.float32)[None, :], (128, 1))
    )
    b3T = np.ascontiguousarray(np.asarray(b3, np.float32).reshape(2, 128).T)
    blr = np.asarray(bl, np.float32).reshape(1, F)

    shared = {
        "W1": W1pk,
        "W2": W2pk,
        "W3": W3pk,
        "Wl": Wlpk,
        "b1": b1T,
        "b2": b2rep,
        "b3": b3T,
        "bl": blr,
    }
    in_maps = []
    for c in range(NCORES):
        m = dict(shared)
        m["xT"] = np.ascontiguousarray(xT_arr[c])
        m["Pm"] = np.ascontiguousarray(P_arr[c])
        m["idx"] = np.ascontiguousarray(idx_arr[c])
        m["sw"] = s_arr[c]
        in_maps.append(m)
    return in_maps, CMAX


def _build(CMAX, phases="ABCDE"):
    """Build the SPMD bass program (identical on all 8 cores).

    phases may carry a "@R" suffix: phases B and D are then emitted R times
    (timing amplification for wall-clock differencing; results unchanged).
    """
    REP = 1
    if "@" in phases:
        phases, r = phases.split("@")
        REP = int(r)
    nc = bacc.Bacc(
        "TRN2",
        target_bir_lowering=False,
        debug=False,
        enable_asserts=False,
        num_devices=NCORES,
    )
    xT_d = nc.dram_tensor("xT", [FPAD, NTP], f16, kind="ExternalInput")
    Pm_d = nc.dram_tensor("Pm", [128, NT * CMAX * 128], f16, kind="ExternalInput")
    idx_d = nc.dram_tensor("idx", [128, NT * CMAX * 8], i16, kind="ExternalInput")
    sw_d = nc.dram_tensor("sw", [128, NT], f16, kind="ExternalInput")
    W1_d = nc.dram_tensor("W1", [128, NFC, H], f16, kind="ExternalInput")
    W2_d = nc.dram_tensor("W2", [128, 2, H], f16, kind="ExternalInput")
    W3_d = nc.dram_tensor("W3", [128, 2, 2, 128], f16, kind="ExternalInput")
    Wl_d = nc.dram_tensor("Wl", [128, 2, F], f16, kind="ExternalInput")
    b1_d = nc.dram_tensor("b1", [128, 2], f32, kind="ExternalInput")
    b2_d = nc.dram_tensor("b2", [128, H], f32, kind="ExternalInput")
    b3_d = nc.dram_tensor("b3", [128, 2], f32, kind="ExternalInput")
    bl_d = nc.dram_tensor("bl", [1, F], f32, kind="ExternalInput")
    out_d = nc.dram_tensor("out", [1, F], f32, kind="ExternalOutput")

    rg = [list(range(NCORES))]
    NSPAN = 4
    SPAN = NTP // NSPAN  # 640 nodes per x^T staging span

    with tile.TileContext(nc) as tc:
        with (
            tc.tile_pool(name="const", bufs=1) as cp,
            tc.tile_pool(name="ff", bufs=3) as fp,
            tc.tile_pool(name="dram", bufs=1, space=bass.MemorySpace.DRAM) as dp,
            tc.tile_pool(name="ps_main", bufs=2, space=bass.MemorySpace.PSUM) as ps_main,
            tc.tile_pool(name="ps_ab", bufs=4, space=bass.MemorySpace.PSUM) as ps_ab,
            tc.tile_pool(name="ps_t", bufs=2, space=bass.MemorySpace.PSUM) as ps_t,
        ):
            # ---- resident constants -> SBUF
            P_sb = cp.tile([128, NT * CMAX * 128], f16)
            nc.sync.dma_start(P_sb[:], Pm_d[:, :])
            idx_sb = cp.tile([128, NT * CMAX * 8], i16)
            nc.sync.dma_start(idx_sb[:], idx_d[:, :])
            s_sb = cp.tile([128, NT], f16)
            nc.sync.dma_start(s_sb[:], sw_d[:, :])
            W1_sb = cp.tile([128, NFC, H], f16)
            nc.sync.dma_start(W1_sb[:], W1_d[:, :, :])
            W2_sb = cp.tile([128, 2, H], f16)
            nc.sync.dma_start(W2_sb[:], W2_d[:, :, :])
            W3_sb = cp.tile([128, 2, 2, 128], f16)
            nc.sync.dma_start(W3_sb[:], W3_d[:, :, :, :])
            Wl_sb = cp.tile([128, 2, F], f16)
            nc.sync.dma_start(Wl_sb[:], Wl_d[:, :, :])
            b1_sb = cp.tile([128, 2], f32)
            nc.sync.dma_start(b1_sb[:], b1_d[:, :])
            b2_sb = cp.tile([128, H], f32)
            nc.sync.dma_start(b2_sb[:], b2_d[:, :])
            b3_sb = cp.tile([128, 2], f32)
            nc.sync.dma_start(b3_sb[:], b3_d[:, :])
            bl_sb = cp.tile([1, F], f32)
            nc.sync.dma_start(bl_sb[:], bl_d[:, :])

            h1T_sb = cp.tile([128, 2, NTP], f16)  # h1^T (features on partitions)

            # DRAM bounce/result buffers
            x1_local = dp.tile([NTP, H], f16)
            x1_full = dp.tile([N, H], f16)
            x2_local = dp.tile([NTP, H], f16)
            x2_full = dp.tile([N, H], f16)
            t_local = dp.tile([128, 2], f32)
            t_red = dp.tile([128, 2], f32)

            # ================= Phase A: x1 = x @ W1 (node-major out) ======
            x1l_ap = x1_local[:, :].rearrange("(t p) f -> p t f", p=128)
            xtp_ctx = tc.tile_pool(name="xts", bufs=2)
            xtp = xtp_ctx.__enter__()
            for sp in range(NSPAN):
                xts = xtp.tile([128, NFC, SPAN], f16)
                nc.sync.dma_start(
                    xts[:], xT_d[:, :].rearrange("(c p) n -> p c n", p=128)[
                        :, :, sp * SPAN : (sp + 1) * SPAN
                    ]
                )
                for tt in range(SPAN // 128):
                    t = sp * (SPAN // 128) + tt
                    ps = ps_main.tile([128, H], f32, tag="ps")
                    for fc in range(NFC):
                        nc.tensor.matmul(
                            ps[:],
                            xts[:, fc, tt * 128 : (tt + 1) * 128],
                            W1_sb[:, fc, :],
                            start=(fc == 0),
                            stop=(fc == NFC - 1),
                        )
                    x1t = fp.tile([128, H], f16, tag="xf")
                    nc.vector.tensor_copy(x1t[:], ps[:])
                    nc.sync.dma_start(x1l_ap[:, t, :], x1t[:])

            xtp_ctx.__exit__(None, None, None)
            gp_ctx = tc.tile_pool(name="gat", bufs=6)
            gp = gp_ctx.__enter__()

            # ================= AllGather x1 ===============================
            if "G" not in phases:  # G = skip collectives (debug)
                nc.gpsimd.collective_compute(
                    "AllGather",
                    mybir.AluOpType.bypass,
                    replica_groups=rg,
                    ins=[x1_local[0:NPC, :].opt()],
                    outs=[x1_full[:, :].opt()],
                )
            else:
                nc.sync.dma_start(x1_full[0:NTP, :], x1_local[:, :])

            # ================= Phase B: conv1 aggregation (h1^T out) ======
            for t in range((NT * REP) if "B" in phases else 0):
                t = t % NT
                gat = gp.tile([128, CMAX, H], f16, tag="gat")
                nc.gpsimd.dma_gather(
                    gat[:],
                    x1_full[:, :],
                    idx_sb[:, t * CMAX * 8 : (t + 1) * CMAX * 8],
                    num_idxs=CMAX * 128,
                    num_idxs_reg=CMAX * 128,
                    elem_size=H,
                    single_packet=False,
                )
                psA = ps_ab.tile([128, 128], f32, tag="psab")
                psB = ps_ab.tile([128, 128], f32, tag="psab")
                for c in range(CMAX):
                    pc = P_sb[:, (t * CMAX + c) * 128 : (t * CMAX + c + 1) * 128]
                    nc.tensor.matmul(
                        psA[:], gat[:, c, 0:128], pc,
                        start=(c == 0), stop=(c == CMAX - 1),
                    )
                    nc.tensor.matmul(
                        psB[:], gat[:, c, 128:256], pc,
                        start=(c == 0), stop=(c == CMAX - 1),
                    )
                nc.scalar.activation(
                    h1T_sb[:, 0, t * 128 : (t + 1) * 128], psA[:],
                    mybir.ActivationFunctionType.Relu, bias=b1_sb[:, 0:1],
                )
                nc.scalar.activation(
                    h1T_sb[:, 1, t * 128 : (t + 1) * 128], psB[:],
                    mybir.ActivationFunctionType.Relu, bias=b1_sb[:, 1:2],
                )

            # ================= Phase C: x2 = h1 @ W2 ======================
            x2l_ap = x2_local[:, :].rearrange("(t p) f -> p t f", p=128)
            for t in range(NT if "C" in phases else 0):
                ps = ps_main.tile([128, H], f32, tag="ps")
                for hh in range(2):
                    nc.tensor.matmul(
                        ps[:],
                        h1T_sb[:, hh, t * 128 : (t + 1) * 128],
                        W2_sb[:, hh, :],
                        start=(hh == 0),
                        stop=(hh == 1),
                    )
                x2t = fp.tile([128, H], f16, tag="xf")
                nc.vector.tensor_copy(x2t[:], ps[:])
                nc.sync.dma_start(x2l_ap[:, t, :], x2t[:])

            # ================= AllGather x2 ===============================
            if "D" in phases and "G" not in phases:
                nc.gpsimd.collective_compute(
                    "AllGather",
                    mybir.AluOpType.bypass,
                    replica_groups=rg,
                    ins=[x2_local[0:NPC, :].opt()],
                    outs=[x2_full[:, :].opt()],
                )
            elif "D" in phases:
                nc.sync.dma_start(x2_full[0:NTP, :], x2_local[:, :])

            # ====== Phase D: conv2 aggregation (node-major) + s-fold ======
            tpsA = ps_t.tile([128, 1], f32, tag="tps")
            tpsB = ps_t.tile([128, 1], f32, tag="tps")
            for tr in range((NT * REP) if "D" in phases else 0):
                rep, t = divmod(tr, NT)
                gat = gp.tile([128, CMAX, H], f16, tag="gat")
                nc.gpsimd.dma_gather(
                    gat[:],
                    x2_full[:, :],
                    idx_sb[:, t * CMAX * 8 : (t + 1) * CMAX * 8],
                    num_idxs=CMAX * 128,
                    num_idxs_reg=CMAX * 128,
                    elem_size=H,
                    single_packet=False,
                )
                ps2 = ps_main.tile([128, H], f32, tag="ps")
                for c in range(CMAX):
                    nc.tensor.matmul(
                        ps2[:],
                        P_sb[:, (t * CMAX + c) * 128 : (t * CMAX + c + 1) * 128],
                        gat[:, c, :],
                        start=(c == 0),
                        stop=(c == CMAX - 1),
                    )
                h2t = fp.tile([128, H], f16, tag="h2")
                nc.vector.tensor_tensor(h2t[:], ps2[:], b2_sb[:], mybir.AluOpType.add)
                nc.vector.tensor_relu(h2t[:], h2t[:])
                if rep != REP - 1:
                    continue
                # t += h2_tile^T @ s_tile  (accumulated across all 20 tiles)
                nc.tensor.matmul(
                    tpsA[:], h2t[:, 0:128], s_sb[:, t : t + 1],
                    start=(t == 0), stop=(t == NT - 1), skip_group_check=True,
                )
                nc.tensor.matmul(
                    tpsB[:], h2t[:, 128:256], s_sb[:, t : t + 1],
                    start=(t == 0), stop=(t == NT - 1), skip_group_check=True,
                )

            gp_ctx.__exit__(None, None, None)

            # ================= Phase E: head ==============================
            if "E" not in phases:
                outsb0 = cp.tile([1, F], f32)
                nc.gpsimd.memset(outsb0[:], 0.0)
                nc.sync.dma_start(out_d[:, :], outsb0[:])
            else:
                tsb = cp.tile([128, 2], f32)
                nc.vector.tensor_copy(tsb[:, 0:1], tpsA[:])
                nc.vector.tensor_copy(tsb[:, 1:2], tpsB[:])
                nc.sync.dma_start(t_local[:, :], tsb[:])
                if "G" not in phases:
                    nc.gpsimd.collective_compute(
                        "AllReduce",
                        mybir.AluOpType.add,
                        replica_groups=rg,
                        ins=[t_local[:, :].opt()],
                        outs=[t_red[:, :].opt()],
                    )
                else:
                    nc.sync.dma_start(t_red[:, :], t_local[:, :])
                t16 = cp.tile([128, 2], f16)
                nc.gpsimd.dma_start(t16[:], t_red[:, :])  # SWDGE cast f32->f16

                g16 = cp.tile([128, 2], f16)
                for o in range(2):
                    psg = ps_t.tile([128, 1], f32, tag="tps")
                    for i in range(2):
                        nc.tensor.matmul(
                            psg[:], W3_sb[:, i, o, :], t16[:, i : i + 1],
                            start=(i == 0), stop=(i == 1),
                        )
                    # g = psg / N + b3  (cast to fp16)
                    nc.vector.tensor_scalar(
                        g16[:, o : o + 1], psg[:], 1.0 / N, b3_sb[:, o : o + 1],
                        mybir.AluOpType.mult, mybir.AluOpType.add,
                    )

                outsb = cp.tile([1, F], f32)
                off = 0
                while off < F:
                    w = min(512, F - off)
                    pso = ps_main.tile([1, 512], f32, tag="ps")
                    for hh in range(2):
                        nc.tensor.matmul(
                            pso[0:1, 0:w], g16[:, hh : hh + 1],
                            Wl_sb[:, hh, off : off + w],
                            start=(hh == 0), stop=(hh == 1),
                        )
                    nc.vector.tensor_tensor(
                        outsb[0:1, off : off + w], pso[0:1, 0:w],
                        bl_sb[0:1, off : off + w], mybir.AluOpType.add,
                    )
                    off += w
                nc.sync.dma_start(out_d[:, :], outsb[:])

    nc.compile()
    return nc


_CACHE = {}


def _get_nc(CMAX, phases="ABCDE"):
    key = (CMAX, phases)
    if key not in _CACHE:
        _CACHE[key] = _build(CMAX, phases)
    return _CACHE[key]


def kernel(x, edge_index, edge_attr, W1, b1, W2, b2, W3, b3, Wl, bl, **kw):
    in_maps, CMAX = _prep(x, edge_index, edge_attr, W1, b1, W2, b2, W3, b3, Wl, bl)
    nc = _get_nc(CMAX)
    res = bass_utils.run_bass_kernel_spmd(nc, in_maps, core_ids=list(range(NCORES)))
    return np.asarray(res.results[0]["out"], np.float32).reshape(1, F)


if __name__ == "__main__":
    import reference

    inputs = {k: np.asarray(v) for k, v in reference.setup_inputs().items()}
    out = kernel(**inputs)
    print("out", out.shape, out.dtype, out[0, :5])


# revision 25
# speedup vs baseline: 2.2605x; 1.1173x over previous
"""Trainium2 8-core GCN kernel (nn_Net_171798692309).

3-layer GCNConv (1700->256->256->256) + global mean pool + linear head.

Strategy:
  - Nodes sharded contiguously across 8 cores (2500/core, padded to 2560).
  - Edges (incl. self-loops) partitioned by TARGET node; per 128-target-node
    tile, edge lists padded to CMAX chunks of 128 edges.
  - x@W1 computed per-core from a host-pretransposed fp16 x^T slice.
  - Feature rows exchanged via AllGather (the "halo exchange" - random graph
    => halo == everything).
  - Per conv: dma_gather pulls 128-row message blocks [128e, 256f] from the
    all-gathered features in HBM; scatter-add is a one-hot matmul
    (P[e, n] = norm_e at column (target-base)) accumulating in PSUM.
  - Conv3 collapses algebraically: its output only feeds a global mean, so
    mean(conv3(h2)) == (s @ h2) @ W3 / N + b3 with s[n] = sum of norm over
    edges with source n. No third gather/scatter.
  - fp16 storage / fp32 accumulation (host-validated rel err ~5e-4).
"""

import sys

import numpy as np

sys.path.insert(0, "/opt/trn_rl_repo")

from concourse import bacc, bass, bass_utils, mybir, tile  # noqa: E402

N = 20000
F = 1700
FPAD = 1792  # 14*128
H = 256
NCORES = 8
NPC = N // NCORES  # 2500 nodes per core
NT = 20  # 128-node tiles per core (last tile: 68 real nodes)
NTP = NT * 128  # 2560 padded nodes per core
NFC = FPAD // 128  # 14 f-chunks for layer 1

f16 = mybir.dt.float16
f32 = mybir.dt.float32
i16 = mybir.dt.int16


def _prep(x, edge_index, edge_attr, W1, b1, W2, b2, W3, b3, Wl, bl):
    """Host-side graph preprocessing + input packing. Returns (in_maps, CMAX)."""
    x = np.asarray(x, np.float32)
    ei = np.asarray(edge_index).astype(np.int64)
    ew = np.asarray(edge_attr, np.float32)
    row, col = ei[0], ei[1]
    E = row.shape[0]

    # symmetric-normalized adjacency weights (same math as reference gcn_conv)
    deg = np.zeros(N, np.float32)
    np.add.at(deg, col, ew)
    deg += 1.0  # self loop weight
    dis = (1.0 / np.sqrt(deg)).astype(np.float32)
    loops = np.arange(N, dtype=np.int64)
    r_all = np.concatenate([row, loops])
    c_all = np.concatenate([col, loops])
    n_all = np.concatenate(
        [(dis[row] * ew * dis[col]).astype(np.float32), (1.0 / deg).astype(np.float32)]
    )
    # s[n] = sum of norm over edges with source n (for collapsed conv3)
    s = np.zeros(N, np.float32)
    np.add.at(s, r_all, n_all)

    # partition edges by target core / target tile
    core = c_all // NPC
    tl = (c_all % NPC) // 128
    coff = (c_all % NPC) % 128
    gid = (core * NT + tl).astype(np.int64)
    order = np.lexsort((r_all, gid))  # by tile, then by source (HBM locality)
    gid_s = gid[order]
    r_s = r_all[order]
    n_s = n_all[order]
    coff_s = coff[order]
    counts = np.bincount(gid_s, minlength=NCORES * NT)
    CMAX = max(1, int(np.ceil(counts.max() / 128.0)))
    starts = np.concatenate([[0], np.cumsum(counts)[:-1]])
    j = np.arange(E + N) - starts[gid_s]  # rank within (core,tile)
    core_s = gid_s // NT
    t_s = gid_s % NT

    # gather index arrays: item j of tile t lives at [j%16, t*CMAX*8 + j//16],
    # replicated across the 8 Q7 cores (partition stripes 16k..16k+15)
    idx16 = np.zeros((NCORES, 16, NT * CMAX * 8), np.int16)
    idx16[core_s, j % 16, t_s * (CMAX * 8) + j // 16] = r_s.astype(np.int16)
    idx_arr = np.tile(idx16, (1, 8, 1))
    # one-hot norm matrices: P[p=j%128, (t*CMAX + j//128)*128 + coff] = norm
    P_arr = np.zeros((NCORES, 128, NT * CMAX * 128), np.float16)
    P_arr[core_s, j % 128, (t_s * CMAX + j // 128) * 128 + coff_s] = n_s.astype(
        np.float16
    )

    # s packed [core, p, t]
    s_pad = np.zeros(NCORES * NTP, np.float32)
    s_pad.reshape(NCORES, NTP)[:, :NPC] = s.reshape(NCORES, NPC)
    s_arr = np.ascontiguousarray(
        s_pad.reshape(NCORES, NT, 128).transpose(0, 2, 1).astype(np.float16)
    )

    # per-core x^T slices, padded [FPAD, NTP] fp16
    xT_arr = np.zeros((NCORES, FPAD, NTP), np.float16)
    for c in range(NCORES):
        xc = x[c * NPC : (c + 1) * NPC]  # [2500, 1700]
        xT_arr[c, :F, :NPC] = xc.T.astype(np.float16)

    W1p = np.zeros((FPAD, H), np.float32)
    W1p[:F] = np.asarray(W1, np.float32)
    W1pk = np.ascontiguousarray(
        W1p.reshape(NFC, 128, H).transpose(1, 0, 2).astype(np.float16)
    )
    W2pk = np.ascontiguousarray(
        np.asarray(W2, np.float32).reshape(2, 128, H).transpose(1, 0, 2).astype(np.float16)
    )
    W3pk = np.ascontiguousarray(
        np.asarray(W3, np.float32)
        .reshape(2, 128, 2, 128)
        .transpose(1, 0, 2, 3)
        .astype(np.float16)
    )
    Wlpk = np.ascontiguousarray(
        np.asarray(Wl, np.float32).reshape(2, 128, F).transpose(1, 0, 2).astype(np.float16)
    )
    b1T = np.ascontiguousarray(np.asarray(b1, np.float32).reshape(2, 128).T)
    b2rep = np.ascontiguousarray(
        np.tile(np.asarray(b2, np.float32)[None, :], (128, 1))
    )
    b3T = np.ascontiguousarray(np.asarray(b3, np.float32).reshape(2, 128).T)
    blr = np.asarray(bl, np.float32).reshape(1, F)

    shared = {
        "W1": W1pk,
        "W2": W2pk,
        "W3": W3pk,
        "Wl": Wlpk,
        "b1": b1T,
        "b2": b2rep,
        "b3": b3T,
        "bl": blr,
    }
    in_maps = []
    for c in range(NCORES):
        m = dict(shared)
        m["xT"] = np.ascontiguousarray(xT_arr[c])
        m["Pm"] = np.ascontiguousarray(P_arr[c])
        m["idx"] = np.ascontiguousarray(idx_arr[c])
        m["sw"] = s_arr[c]
        in_maps.append(m)
    return in_maps, CMAX


def _build(CMAX, phases="ABCDE"):
    """Build the SPMD bass program (identical on all 8 cores).

    phases may carry a "@R" suffix: phases B and D are then emitted R times
    (timing amplification for wall-clock differencing; results unchanged).
    """
    REP = 1
    if "@" in phases:
        phases, r = phases.split("@")
        REP = int(r)
    nc = bacc.Bacc(
        "TRN2",
        target_bir_lowering=False,
        debug=False,
        enable_asserts=False,
        num_devices=NCORES,
    )
    xT_d = nc.dram_tensor("xT", [FPAD, NTP], f16, kind="ExternalInput")
    Pm_d = nc.dram_tensor("Pm", [128, NT * CMAX * 128], f16, kind="ExternalInput")
    idx_d = nc.dram_tensor("idx", [128, NT * CMAX * 8], i16, kind="ExternalInput")
    sw_d = nc.dram_tensor("sw", [128, NT], f16, kind="ExternalInput")
    W1_d = nc.dram_tensor("W1", [128, NFC, H], f16, kind="ExternalInput")
    W2_d = nc.dram_tensor("W2", [128, 2, H], f16, kind="ExternalInput")
    W3_d = nc.dram_tensor("W3", [128, 2, 2, 128], f16, kind="ExternalInput")
    Wl_d = nc.dram_tensor("Wl", [128, 2, F], f16, kind="ExternalInput")
    b1_d = nc.dram_tensor("b1", [128, 2], f32, kind="ExternalInput")
    b2_d = nc.dram_tensor("b2", [128, H], f32, kind="ExternalInput")
    b3_d = nc.dram_tensor("b3", [128, 2], f32, kind="ExternalInput")
    bl_d = nc.dram_tensor("bl", [1, F], f32, kind="ExternalInput")
    out_d = nc.dram_tensor("out", [1, F], f32, kind="ExternalOutput")

    rg = [list(range(NCORES))]
    NSPAN = 4
    SPAN = NTP // NSPAN  # 640 nodes per x^T staging span

    with tile.TileContext(nc) as tc:
        with (
            tc.tile_pool(name="const", bufs=1) as cp,
            tc.tile_pool(name="ff", bufs=3) as fp,
            tc.tile_pool(name="dram", bufs=1, space=bass.MemorySpace.DRAM) as dp,
            tc.tile_pool(name="ps_main", bufs=2, space=bass.MemorySpace.PSUM) as ps_main,
            tc.tile_pool(name="ps_ab", bufs=4, space=bass.MemorySpace.PSUM) as ps_ab,
            tc.tile_pool(name="ps_t", bufs=2, space=bass.MemorySpace.PSUM) as ps_t,
        ):
            # ---- resident constants -> SBUF
            P_sb = cp.tile([128, NT * CMAX * 128], f16)
            nc.sync.dma_start(P_sb[:], Pm_d[:, :])
            idx_sb = cp.tile([128, NT * CMAX * 8], i16)
            nc.sync.dma_start(idx_sb[:], idx_d[:, :])
            s_sb = cp.tile([128, NT], f16)
            nc.sync.dma_start(s_sb[:], sw_d[:, :])
            W1_sb = cp.tile([128, NFC, H], f16)
            nc.sync.dma_start(W1_sb[:], W1_d[:, :, :])
            W2_sb = cp.tile([128, 2, H], f16)
            nc.sync.dma_start(W2_sb[:], W2_d[:, :, :])
            W3_sb = cp.tile([128, 2, 2, 128], f16)
            nc.sync.dma_start(W3_sb[:], W3_d[:, :, :, :])
            Wl_sb = cp.tile([128, 2, F], f16)
            nc.sync.dma_start(Wl_sb[:], Wl_d[:, :, :])
            b1_sb = cp.tile([128, 2], f32)
            nc.sync.dma_start(b1_sb[:], b1_d[:, :])
            b2_sb = cp.tile([128, H], f32)
            nc.sync.dma_start(b2_sb[:], b2_d[:, :])
            b3_sb = cp.tile([128, 2], f32)
            nc.sync.dma_start(b3_sb[:], b3_d[:, :])
            bl_sb = cp.tile([1, F], f32)
            nc.sync.dma_start(bl_sb[:], bl_d[:, :])

            h1T_sb = cp.tile([128, 2, NTP], f16)  # h1^T (features on partitions)

            # DRAM bounce/result buffers
            x1_local = dp.tile([NTP, H], f16)
            x1_full = dp.tile([N, H], f16)
            x2_local = dp.tile([NTP, H], f16)
            x2_full = dp.tile([N, H], f16)
            t_local = dp.tile([128, 2], f32)
            t_red = dp.tile([128, 2], f32)

            # ================= Phase A: x1 = x @ W1 (node-major out) ======
            x1l_ap = x1_local[:, :].rearrange("(t p) f -> p t f", p=128)
            xtp_ctx = tc.tile_pool(name="xts", bufs=2)
            xtp = xtp_ctx.__enter__()
            for sp in range(NSPAN):
                xts = xtp.tile([128, NFC, SPAN], f16)
                nc.sync.dma_start(
                    xts[:], xT_d[:, :].rearrange("(c p) n -> p c n", p=128)[
                        :, :, sp * SPAN : (sp + 1) * SPAN
                    ]
                )
                for tt in range(SPAN // 128):
                    t = sp * (SPAN // 128) + tt
                    ps = ps_main.tile([128, H], f32, tag="ps")
                    for fc in range(NFC):
                        nc.tensor.matmul(
                            ps[:],
                            xts[:, fc, tt * 128 : (tt + 1) * 128],
                            W1_sb[:, fc, :],
                            start=(fc == 0),
                            stop=(fc == NFC - 1),
                        )
                    x1t = fp.tile([128, H], f16, tag="xf")
                    nc.vector.tensor_copy(x1t[:], ps[:])
                    nc.sync.dma_start(x1l_ap[:, t, :], x1t[:])

            xtp_ctx.__exit__(None, None, None)
            gp_ctx = tc.tile_pool(name="gat", bufs=6)
            gp = gp_ctx.__enter__()

            # ================= AllGather x1 ===============================
            if "G" not in phases:  # G = skip collectives (debug)
                nc.gpsimd.collective_compute(
                    "AllGather",
                    mybir.AluOpType.bypass,
                    replica_groups=rg,
                    ins=[x1_local[0:NPC, :].opt()],
                    outs=[x1_full[:, :].opt()],
                )
            else:
                nc.sync.dma_start(x1_full[0:NTP, :], x1_local[:, :])

            # ================= Phase B: conv1 aggregation (h1^T out) ======
            for t in range((NT * REP) if "B" in phases else 0):
                t = t % NT
                gat = gp.tile([128, CMAX, H], f16, tag="gat")
                nc.gpsimd.dma_gather(
                    gat[:],
                    x1_full[:, :],
                    idx_sb[:, t * CMAX * 8 : (t + 1) * CMAX * 8],
                    num_idxs=CMAX * 128,
                    num_idxs_reg=CMAX * 128,
                    elem_size=H,
                    single_packet=False,
                )
                psA = ps_ab.tile([128, 128], f32, tag="psab")
                psB = ps_ab.tile([128, 128], f32, tag="psab")
                for c in range(CMAX):
                    pc = P_sb[:, (t * CMAX + c) * 128 : (t * CMAX + c + 1) * 128]
                    nc.tensor.matmul(
                        psA[:], gat[:, c, 0:128], pc,
                        start=(c == 0), stop=(c == CMAX - 1),
                    )
                    nc.tensor.matmul(
                        psB[:], gat[:, c, 128:256], pc,
                        start=(c == 0), stop=(c == CMAX - 1),
                    )
                nc.scalar.activation(
                    h1T_sb[:, 0, t * 128 : (t + 1) * 128], psA[:],
                    mybir.ActivationFunctionType.Relu, bias=b1_sb[:, 0:1],
                )
                nc.scalar.activation(
                    h1T_sb[:, 1, t * 128 : (t + 1) * 128], psB[:],
                    mybir.ActivationFunctionType.Relu, bias=b1_sb[:, 1:2],
                )

            # ================= Phase C: x2 = h1 @ W2 ======================
            x2l_ap = x2_local[:, :].rearrange("(t p) f -> p t f", p=128)
            for t in range(NT if "C" in phases else 0):
                ps = ps_main.tile([128, H], f32, tag="ps")
                for hh in range(2):
                    nc.tensor.matmul(
                        ps[:],
                        h1T_sb[:, hh, t * 128 : (t + 1) * 128],
                        W2_sb[:, hh, :],
                        start=(hh == 0),
                        stop=(hh == 1),
                    )
                x2t = fp.tile([128, H], f16, tag="xf")
                nc.vector.tensor_copy(x2t[:], ps[:])
                nc.sync.dma_start(x2l_ap[:, t, :], x2t[:])

            # ================= AllGather x2 ===============================
            if "D" in phases and "G" not in phases:
                nc.gpsimd.collective_compute(
                    "AllGather",
                    mybir.AluOpType.bypass,
                    replica_groups=rg,
                    ins=[x2_local[0:NPC, :].opt()],
                    outs=[x2_full[:, :].opt()],
                )
            elif "D" in phases:
                nc.sync.dma_start(x2_full[0:NTP, :], x2_local[:, :])

            # ====== Phase D: conv2 aggregation (node-major) + s-fold ======
            tpsA = ps_t.tile([128, 1], f32, tag="tps")
            tpsB = ps_t.tile([128, 1], f32, tag="tps")
            for tr in range((NT * REP) if "D" in phases else 0):
                rep, t = divmod(tr, NT)
                gat = gp.tile([128, CMAX, H], f16, tag="gat")
                nc.gpsimd.dma_gather(
                    gat[:],
                    x2_full[:, :],
                    idx_sb[:, t * CMAX * 8 : (t + 1) * CMAX * 8],
                    num_idxs=CMAX * 128,
                    num_idxs_reg=CMAX * 128,
                    elem_size=H,
                    single_packet=False,
                )
                ps2 = ps_main.tile([128, H], f32, tag="ps")
                for c in range(CMAX):
                    nc.tensor.matmul(
                        ps2[:],
                        P_sb[:, (t * CMAX + c) * 128 : (t * CMAX + c + 1) * 128],
                        gat[:, c, :],
                        start=(c == 0),
                        stop=(c == CMAX - 1),
                    )
                h2t = fp.tile([128, H], f16, tag="h2")
                nc.vector.tensor_tensor(h2t[:], ps2[:], b2_sb[:], mybir.AluOpType.add)
                nc.vector.tensor_relu(h2t[:], h2t[:])
                if rep != REP - 1:
                    continue
                # t += h2_tile^T @ s_tile  (accumulated across all 20 tiles)
                nc.tensor.matmul(
                    tpsA[:], h2t[:, 0:128], s_sb[:, t : t + 1],
                    start=(t == 0), stop=(t == NT - 1), skip_group_check=True,
                )
                nc.tensor.matmul(
                    tpsB[:], h2t[:, 128:256], s_sb[:, t : t + 1],
                    start=(t == 0), stop=(t == NT - 1), skip_group_check=True,
                )

            gp_ctx.__exit__(None, None, None)

            # ================= Phase E: head ==============================
            if "E" not in phases:
                outsb0 = cp.tile([1, F], f32)
                nc.gpsimd.memset(outsb0[:], 0.0)
                nc.sync.dma_start(out_d[:, :], outsb0[:])
            else:
                tsb = cp.tile([128, 2], f32)
                nc.vector.tensor_copy(tsb[:, 0:1], tpsA[:])
                nc.vector.tensor_copy(tsb[:, 1:2], tpsB[:])
                nc.sync.dma_start(t_local[:, :], tsb[:])
                if "G" not in phases:
                    nc.gpsimd.collective_compute(
                        "AllReduce",
                        mybir.AluOpType.add,
                        replica_groups=rg,
                        ins=[t_local[:, :].opt()],
                        outs=[t_red[:, :].opt()],
                    )
                else:
                    nc.sync.dma_start(t_red[:, :], t_local[:, :])
                t16 = cp.tile([128, 2], f16)
                nc.gpsimd.dma_start(t16[:], t_red[:, :])  # SWDGE cast f32->f16

                g16 = cp.tile([128, 2], f16)
                for o in range(2):
                    psg = ps_t.tile([128, 1], f32, tag="tps")
                    for i in range(2):
                        nc.tensor.matmul(
                            psg[:], W3_sb[:, i, o, :], t16[:, i : i + 1],
                            start=(i == 0), stop=(i == 1),
                        )
                    # g = psg / N + b3  (cast to fp16)
                    nc.vector.tensor_scalar(
                        g16[:, o : o + 1], psg[:], 1.0 / N, b3_sb[:, o : o + 1],
                        mybir.AluOpType.mult, mybir.AluOpType.add,
                    )

                outsb = cp.tile([1, F], f32)
                off = 0
                while off < F:
                    w = min(512, F - off)
                    pso = ps_main.tile([1, 512], f32, tag="ps")
                    for hh in range(2):
                        nc.tensor.matmul(
                            pso[0:1, 0:w], g16[:, hh : hh + 1],
                            Wl_sb[:, hh, off : off + w],
                            start=(hh == 0), stop=(hh == 1),
                        )
                    nc.vector.tensor_tensor(
                        outsb[0:1, off : off + w], pso[0:1, 0:w],
                        bl_sb[0:1, off : off + w], mybir.AluOpType.add,
                    )
                    off += w
                nc.sync.dma_start(out_d[:, :], outsb[:])

    nc.compile()
    return nc


_CACHE = {}


def _get_nc(CMAX, phases="ABCDE"):
    key = (CMAX, phases)
    if key not in _CACHE:
        _CACHE[key] = _build(CMAX, phases)
    return _CACHE[key]


def kernel(x, edge_index, edge_attr, W1, b1, W2, b2, W3, b3, Wl, bl, **kw):
    in_maps, CMAX = _prep(x, edge_index, edge_attr, W1, b1, W2, b2, W3, b3, Wl, bl)
    nc = _get_nc(CMAX)
    res = bass_utils.run_bass_kernel_spmd(nc, in_maps, core_ids=list(range(NCORES)))
    return np.asarray(res.results[0]["out"], np.float32).reshape(1, F)


if __name__ == "__main__":
    import reference

    inputs = {k: np.asarray(v) for k, v in reference.setup_inputs().items()}
    out = kernel(**inputs)
    print("out", out.shape, out.dtype, out[0, :5])
